# revision 10
# baseline (speedup 1.0000x reference)
"""Trainium2 Bass kernel for CrossModalAttention (attention + residual + LayerNorm).

Math: the reference concatenates [query, key_value], projects Q/K/V, attends with an
additive -10000 mask on key columns < N_q, and keeps only the query-row outputs.
With scores ~ N(0,1), exp(score - 10000 - rowmax) underflows to exactly 0 in fp32,
so the computation is exactly standard cross-attention:
    Q = query @ Wq + bq ; K = key_value @ Wk + bk ; V = key_value @ Wv + bv
    C = softmax(Q K^T / 8) V ;  out = LayerNorm(query + C) * gamma + beta

Additional exact simplifications used here:
  - bk drops entirely: (q+bq).(k+bk) differs from (q+bq).k by a per-query constant,
    which softmax cancels.
  - bv folds into the residual: softmax rows sum to 1, so ctx(V+bv) = ctx(V) + bv;
    the host adds bv to the residual input.

Sharding: 8 cores = 2 batches x 4 query-blocks of 512 rows. Each core computes the
K/V projections for its batch (duplicated across the 4 cores of a batch) and full
8-head attention + LN for its query block.

Per-core kernel (v5):
  - All projections run up-front, pipelined with DMA arrival (kvT split in four
    key-blocks, loads spread over four DMA queues); the attention loop then runs
    pure scores/exp/PV at the softmax-exp pace.
  - Q/K projections in bf16; V projection in fp8e4 DoubleRow (2 MACs/cell).
  - scores S^T[k, q] per head via K=64 matmuls, two heads packed with
    tile_position row tiling (bf16 operands).
  - softmax exp split across engines: ScalarE runs true Exp (scale=1/8 folded),
    VectorE runs Schraudolph fast-exp (int16(x*184.665/8 + B) bitcast to bf16,
    ~+-4% relative, cancels in softmax renorm; final output error ~0.7%).
  - PV with ones-augmented V (M=65): row 64 accumulates the softmax denominator.
  - PE-transpose C^T -> C (both heads of a pair into one PSUM tile), normalize
    by 1/l, residual + LayerNorm on Vector/Scalar.
"""

import os
import sys

import numpy as np
import ml_dtypes

try:
    import concourse.bass as bass  # noqa: F401
except ImportError:
    for _p in ("/opt/trn_rl_repo", "/root/.axon_site/_ro/trn_rl_repo"):
        if os.path.isdir(_p):
            sys.path.insert(0, _p)
            break
    import concourse.bass as bass  # noqa: F401

import concourse.tile as tile
from concourse import bacc, bass_utils, mybir
from concourse.masks import make_identity

F32 = mybir.dt.float32
BF16 = mybir.dt.bfloat16
I16 = mybir.dt.int16
F8 = mybir.dt.float8e4
AF = mybir.ActivationFunctionType
ALU = mybir.AluOpType
DR = mybir.MatmulPerfMode.DoubleRow

B, N_Q, N_KV, D, H, HD = 2, 2048, 2048, 512, 8, 64
N_CORES = 8
QBLK = N_Q // 4          # 512 query rows per core
DC = D // 128            # 4 partition chunks of the model dim
KC = N_KV // 128         # 16 key chunks
NB = N_KV // 512         # 4 key blocks (DMA/projection granularity)
LN_EPS = 1e-5
SM_SCALE = 1.0 / np.sqrt(HD)

# Schraudolph fast-exp constants (bf16 bit trick): bf16 bits of ~exp(s/8) are
# int16(s * A + Bc). Renormalization cancels the piecewise-linear bias.
SCH_A = float((2.0 ** 7) / np.log(2.0) / 8.0)
SCH_B = float(127.0 * (2 ** 7) - 6.0)
# kc tiles with (kc % 16) < SPLIT go to VectorE fast-exp, rest to ScalarE Exp.
SPLIT = 6


def _build_program(trace=False):
    nc = bacc.Bacc("TRN2", target_bir_lowering=False, debug=False,
                   num_devices=N_CORES)

    def din(name, shape, dt):
        return nc.dram_tensor(name, shape, dt, kind="ExternalInput").ap()

    xqT = din("xqT", [128, DC, QBLK], BF16)   # query block^T, chunk layout
    kvT = din("kvT", [128, DC, N_KV], BF16)   # key_value^T, chunk layout
    wq = din("wq", [128, DC, D], BF16)        # weights, chunk layout
    wk = din("wk", [128, DC, D], BF16)
    kvt8 = din("kvt8", [128, DC, N_KV], F8)   # key_value^T, DR subtile layout
    wv8 = din("wv8", [128, DC, D], F8)        # Wv, DR subtile layout
    xqr = din("xqr", [QBLK, D], F32)          # query block + bv (residual base)
    bqc = din("bqc", [128, DC], F32)          # bq chunked [128, 4]
    gammab = din("gammab", [128, D], F32)
    betab = din("betab", [128, D], F32)
    out = nc.dram_tensor("out", [QBLK, D], F32, kind="ExternalOutput").ap()

    with tile.TileContext(nc) as tc:
        with (
            tc.tile_pool(name="persist", bufs=1) as pp,
            tc.tile_pool(name="work", bufs=2) as wkp,
            tc.tile_pool(name="small", bufs=8) as smp,
            tc.tile_pool(name="scratch_ps", bufs=2, space="PSUM") as sps,
            tc.tile_pool(name="sc_ps", bufs=2, space="PSUM") as scps,
            tc.tile_pool(name="pv_ps", bufs=1, space="PSUM") as pvps,
        ):
            # ---- persistent SBUF tiles ----
            wq_sb = pp.tile([128, DC, D], BF16, name="wq", tag="wq")
            wk_sb = pp.tile([128, DC, D], BF16, name="wk", tag="wk")
            xqt_sb = pp.tile([128, DC, QBLK], BF16, name="xqt", tag="xqt")
            kvt_sb = [pp.tile([128, DC, 512], BF16, name=f"kvt{n}", tag=f"kvt{n}")
                      for n in range(NB)]
            wv_sb = pp.tile([128, DC, D], F8, name="wv8", tag="wv8")
            kvt8_sb = [pp.tile([128, DC, N_KV // 2], F8, name=f"kvt8_{i}",
                               tag=f"kvt8_{i}") for i in range(2)]
            xqr_sb = [pp.tile([128, D], F32, name=f"xqr{q}", tag=f"xqr{q}")
                      for q in range(4)]
            bqc_sb = pp.tile([128, DC], F32, name="bqc", tag="bqc")
            gam_sb = pp.tile([128, D], F32, name="gam", tag="gam")
            bet_sb = pp.tile([128, D], F32, name="bet", tag="bet")

            # ---- loads across four DMA queues, critical tensors first ----
            nc.sync.dma_start(bqc_sb[:], bqc)
            nc.sync.dma_start(wq_sb[:], wq)
            nc.sync.dma_start(xqt_sb[:], xqT)
            nc.sync.dma_start(wk_sb[:], wk)
            for n in range(NB):
                eng = (nc.sync, nc.gpsimd, nc.scalar, nc.sync)[n]
                eng.dma_start(kvt_sb[n][:], kvT[:, :, n * 512:(n + 1) * 512])
            nc.gpsimd.dma_start(kvt8_sb[0][:], kvt8[:, :, 0:N_KV // 2])
            nc.scalar.dma_start(kvt8_sb[1][:], kvt8[:, :, N_KV // 2:])
            nc.gpsimd.dma_start(wv_sb[:], wv8)
            for q in range(4):
                nc.gpsimd.dma_start(xqr_sb[q][:], xqr[q * 128:(q + 1) * 128, :])
            nc.gpsimd.dma_start(gam_sb[:], gammab)
            nc.gpsimd.dma_start(bet_sb[:], betab)
            ident = pp.tile([128, 128], F32, name="ident", tag="ident")
            make_identity(nc, ident[:])
            eps_sb = pp.tile([128, 1], F32, name="eps", tag="eps")
            nc.vector.memset(eps_sb[:], float(LN_EPS))

            qt_sb = [pp.tile([128, QBLK], BF16, name=f"qt{m}", tag=f"qt{m}")
                     for m in range(DC)]
            kt_sb = [pp.tile([128, N_KV], BF16, name=f"kt{m}", tag=f"kt{m}")
                     for m in range(DC)]
            vaug_sb = [pp.tile([128, H * (HD + 1)], BF16, name=f"va{t}", tag=f"va{t}")
                       for t in range(KC)]
            c_sb = [pp.tile([128, D], F32, name=f"csb{q}", tag=f"csb{q}")
                    for q in range(4)]

            def proj_qt(m):
                ps = sps.tile([128, QBLK], F32, name="ps_qt", tag="scratch")
                for c in range(DC):
                    nc.tensor.matmul(
                        ps[:], wq_sb[:, c, m * 128:(m + 1) * 128],
                        xqt_sb[:, c, :], start=(c == 0), stop=(c == DC - 1))
                nc.vector.tensor_scalar(
                    out=qt_sb[m][:], in0=ps[:], scalar1=bqc_sb[:, m:m + 1],
                    scalar2=None, op0=ALU.add)

            def proj_kt_n(n):
                # all four output chunks for key-block n (starts as soon as
                # kvt block n lands)
                for m in range(DC):
                    ps = sps.tile([128, 512], F32, name="ps_kt", tag="scratch")
                    for c in range(DC):
                        nc.tensor.matmul(
                            ps[:], wk_sb[:, c, m * 128:(m + 1) * 128],
                            kvt_sb[n][:, c, :],
                            start=(c == 0), stop=(c == DC - 1))
                    nc.scalar.copy(kt_sb[m][:, n * 512:(n + 1) * 512], ps[:])

            def proj_v(t):
                half = kvt8_sb[t // (KC // 2)]
                toff = (t % (KC // 2)) * 128
                ps = sps.tile([128, D], F32, name="ps_v", tag="scratch")
                for cp in range(2):
                    nc.tensor.matmul(
                        ps[:], half[:, 2 * cp:2 * cp + 2, toff:toff + 128],
                        wv_sb[:, 2 * cp:2 * cp + 2, :],
                        start=(cp == 0), stop=(cp == 1), perf_mode=DR)
                va3 = vaug_sb[t][:].rearrange("p (h d) -> p h d", h=H)
                nc.vector.tensor_scalar(
                    out=va3[:, :, 0:HD],
                    in0=ps[:].rearrange("p (h d) -> p h d", h=H),
                    scalar1=0.0, scalar2=None, op0=ALU.add)
                nc.gpsimd.memset(vaug_sb[t][:, HD::HD + 1], 1.0)

            def scores_exp(g, kc):
                psc = scps.tile([128, 2 * QBLK], F32, name="psc", tag="sc")
                for j in range(2):
                    nc.tensor.matmul(
                        psc[:, j * QBLK:(j + 1) * QBLK],
                        kt_sb[g][j * 64:(j + 1) * 64, kc * 128:(kc + 1) * 128],
                        qt_sb[g][j * 64:(j + 1) * 64, :],
                        start=True, stop=True, tile_position=(j * 64, 0))
                pt = wkp.tile([128, 2 * QBLK], I16, name="pt", tag="pt")
                if (kc % KC) < SPLIT:
                    nc.vector.tensor_scalar(
                        out=pt[:], in0=psc[:], scalar1=float(SCH_A),
                        scalar2=float(SCH_B), op0=ALU.mult, op1=ALU.add)
                else:
                    nc.scalar.activation(pt[:].bitcast(BF16), psc[:],
                                         AF.Exp, scale=float(SM_SCALE))
                return pt

            def pv(g, kc, ppv, pt):
                ptb = pt[:].bitcast(BF16)
                for j in range(2):
                    h = 2 * g + j
                    nc.tensor.matmul(
                        ppv[j][:],
                        vaug_sb[kc][:, h * (HD + 1):(h + 1) * (HD + 1)],
                        ptb[:, j * QBLK:(j + 1) * QBLK],
                        start=(kc == 0), stop=(kc == KC - 1))

            def finish_pair(g, ppv, then_ln=False):
                cts = []
                for j in range(2):
                    ct = wkp.tile([HD + 1, QBLK], F32, name="ct", tag="ct")
                    nc.vector.tensor_copy(ct[:], ppv[j][:])
                    cts.append(ct)
                for q in range(4):
                    # both heads of the pair transpose into one PSUM tile
                    ptr = sps.tile([128, 2 * (HD + 1)], F32, name="ptr",
                                   tag="scratch")
                    for j in range(2):
                        nc.tensor.transpose(
                            ptr[:, j * (HD + 1):(j + 1) * (HD + 1)],
                            cts[j][:, q * 128:(q + 1) * 128],
                            ident[0:HD + 1, 0:HD + 1])
                    linv = smp.tile([128, 2], F32, name="linv", tag="linv")
                    nc.vector.reciprocal(linv[:], ptr[:, HD::HD + 1])
                    for j in range(2):
                        h = 2 * g + j
                        nc.vector.tensor_scalar(
                            out=c_sb[q][:, h * HD:(h + 1) * HD],
                            in0=ptr[:, j * (HD + 1):j * (HD + 1) + HD],
                            scalar1=linv[:, j:j + 1], scalar2=None,
                            op0=ALU.mult)
                    if then_ln:
                        layer_norm(q)

            # ---- residual + LayerNorm ----
            def layer_norm(q):
                resid = wkp.tile([128, D], F32, name="resid", tag="resid")
                rowsum = smp.tile([128, 1], F32, name="rowsum", tag="rowsum")
                nc.vector.scalar_tensor_tensor(
                    out=resid[:], in0=c_sb[q][:], scalar=0.0, in1=xqr_sb[q][:],
                    op0=ALU.bypass, op1=ALU.add, accum_out=rowsum[:])
                sq = wkp.tile([128, D], F32, name="sq", tag="sq")
                sqs = smp.tile([128, 1], F32, name="sqs", tag="sqs")
                nc.scalar.activation(sq[:], resid[:], AF.Square, accum_out=sqs[:])
                mu = smp.tile([128, 1], F32, name="mu", tag="mu")
                nc.vector.tensor_scalar_mul(mu[:], rowsum[:], 1.0 / D)
                musq = smp.tile([128, 1], F32, name="musq", tag="musq")
                nc.vector.tensor_tensor(out=musq[:], in0=mu[:], in1=mu[:], op=ALU.mult)
                var = smp.tile([128, 1], F32, name="var", tag="var")
                nc.vector.scalar_tensor_tensor(
                    out=var[:], in0=sqs[:], scalar=1.0 / D, in1=musq[:],
                    op0=ALU.mult, op1=ALU.subtract)
                std = smp.tile([128, 1], F32, name="std", tag="std")
                nc.scalar.activation(std[:], var[:], AF.Sqrt, bias=eps_sb[:])
                inv = smp.tile([128, 1], F32, name="inv", tag="inv")
                nc.vector.reciprocal(inv[:], std[:])
                xcn = wkp.tile([128, D], F32, name="xcn", tag="xcn")
                nc.vector.tensor_scalar(
                    out=xcn[:], in0=resid[:], scalar1=mu[:], scalar2=inv[:],
                    op0=ALU.subtract, op1=ALU.mult)
                t2 = wkp.tile([128, D], F32, name="t2", tag="t2")
                nc.vector.tensor_tensor(out=t2[:], in0=xcn[:], in1=gam_sb[:],
                                        op=ALU.mult)
                o = wkp.tile([128, D], F32, name="o", tag="o")
                nc.vector.tensor_tensor(out=o[:], in0=t2[:], in1=bet_sb[:],
                                        op=ALU.add)
                nc.sync.dma_start(out[q * 128:(q + 1) * 128, :], o[:])

            # ---- phase schedule ----
            # Phase 0: all projections, pipelined with DMA arrival.
            for m in range(DC):
                proj_qt(m)
            proj_kt_n(0)
            for t in range(0, 4):
                proj_v(t)
            proj_kt_n(1)
            for t in range(4, 8):
                proj_v(t)
            proj_kt_n(2)
            proj_kt_n(3)
            for t in range(8, KC):
                proj_v(t)
            # Phase 1: attention, pure scores/exp/PV; finish of pair g
            # overlaps pair g+1.
            pending = None  # (g, ppv) awaiting finish_pair
            for g in range(DC):
                ppv = [pvps.tile([HD + 1, QBLK], F32, name=f"pv{j}", tag=f"pv{j}")
                       for j in range(2)]
                pt_prev = None
                for kc in range(KC):
                    pt = scores_exp(g, kc)
                    if pt_prev is not None:
                        pv(g, kc - 1, ppv, pt_prev)
                    pt_prev = pt
                pv(g, KC - 1, ppv, pt_prev)
                if pending is not None:
                    finish_pair(*pending)
                pending = (g, ppv)
            finish_pair(*pending, then_ln=True)

    nc.compile()
    return nc


_PROGRAM = None


def _get_program():
    global _PROGRAM
    if _PROGRAM is None:
        _PROGRAM = _build_program()
    return _PROGRAM


def _chunk4(a, dtype):
    """[512, N] -> [128, 4, N] K-subtile/chunk layout."""
    return np.ascontiguousarray(
        np.asarray(a, np.float32).reshape(DC, 128, -1).transpose(1, 0, 2)
    ).astype(dtype)


def _make_in_maps(query, key_value, Wq, bq, Wk, bk, Wv, bv, ln_gamma, ln_beta):
    f32 = np.float32
    f8 = ml_dtypes.float8_e4m3fn
    bf = ml_dtypes.bfloat16
    wq_b = _chunk4(Wq, bf)
    wk_b = _chunk4(Wk, bf)
    wv8 = _chunk4(Wv, f8)
    bqc = np.ascontiguousarray(bq.reshape(DC, 128).T, dtype=f32)
    gammab = np.ascontiguousarray(
        np.broadcast_to(ln_gamma[None, :], (128, D)), dtype=f32)
    betab = np.ascontiguousarray(
        np.broadcast_to(ln_beta[None, :], (128, D)), dtype=f32)
    kvT = [_chunk4(key_value[b].T, bf) for b in range(B)]
    kvt8 = [_chunk4(key_value[b].T, f8) for b in range(B)]
    in_maps = []
    for core in range(N_CORES):
        b, qb = divmod(core, 4)
        blk = query[b, qb * QBLK:(qb + 1) * QBLK, :]
        in_maps.append({
            "xqT": _chunk4(blk.T, bf),
            "xqr": np.ascontiguousarray(blk + bv[None, :], dtype=f32),
            "kvT": kvT[b], "kvt8": kvt8[b],
            "wq": wq_b, "wk": wk_b, "wv8": wv8,
            "bqc": bqc,
            "gammab": gammab, "betab": betab,
        })
    return in_maps


def kernel(query, key_value, Wq, bq, Wk, bk, Wv, bv, ln_gamma, ln_beta,
           _trace=False, _trace_kwargs=None):
    args = [np.asarray(a, dtype=np.float32) for a in
            (query, key_value, Wq, bq, Wk, bk, Wv, bv, ln_gamma, ln_beta)]
    nc = _get_program()
    in_maps = _make_in_maps(*args)
    res = bass_utils.run_bass_kernel_spmd(
        nc, in_maps, core_ids=list(range(N_CORES)), trace=_trace,
        **(_trace_kwargs or {}))
    out = np.empty((B, N_Q, D), np.float32)
    for core in range(N_CORES):
        b, qb = divmod(core, 4)
        out[b, qb * QBLK:(qb + 1) * QBLK, :] = res.results[core]["out"]
    if _trace:
        return out, res
    return out


# revision 14
# speedup vs baseline: 1.0529x; 1.0529x over previous
"""Trainium2 Bass kernel for CrossModalAttention (attention + residual + LayerNorm).

Math: the reference concatenates [query, key_value], projects Q/K/V, attends with an
additive -10000 mask on key columns < N_q, and keeps only the query-row outputs.
With scores ~ N(0,1), exp(score - 10000 - rowmax) underflows to exactly 0 in fp32,
so the computation is exactly standard cross-attention:
    Q = query @ Wq + bq ; K = key_value @ Wk + bk ; V = key_value @ Wv + bv
    C = softmax(Q K^T / 8) V ;  out = LayerNorm(query + C) * gamma + beta

Additional exact simplifications used here:
  - bk drops entirely: (q+bq).(k+bk) differs from (q+bq).k by a per-query constant,
    which softmax cancels.
  - bv folds into the residual: softmax rows sum to 1, so ctx(V+bv) = ctx(V) + bv;
    the host adds bv to the residual input.

Sharding: 8 cores = 2 batches x 4 query-blocks of 512 rows. Each core computes the
K/V projections for its batch (duplicated across the 4 cores of a batch) and full
8-head attention + LN for its query block.

Per-core kernel (v5):
  - All projections run up-front, pipelined with DMA arrival (kvT split in four
    key-blocks, loads spread over four DMA queues); the attention loop then runs
    pure scores/exp/PV at the softmax-exp pace.
  - Q/K projections in bf16; V projection in fp8e4 DoubleRow (2 MACs/cell).
  - scores S^T[k, q] per head via K=64 matmuls, two heads packed with
    tile_position row tiling (bf16 operands).
  - softmax exp split across engines: ScalarE runs true Exp (scale=1/8 folded),
    VectorE runs Schraudolph fast-exp (int16(x*184.665/8 + B) bitcast to bf16,
    ~+-4% relative, cancels in softmax renorm; final output error ~0.7%).
  - PV with ones-augmented V (M=65): row 64 accumulates the softmax denominator.
  - PE-transpose C^T -> C (both heads of a pair into one PSUM tile), normalize
    by 1/l, residual + LayerNorm on Vector/Scalar.
"""

import os
import sys

import numpy as np
import ml_dtypes

try:
    import concourse.bass as bass  # noqa: F401
except ImportError:
    for _p in ("/opt/trn_rl_repo", "/root/.axon_site/_ro/trn_rl_repo"):
        if os.path.isdir(_p):
            sys.path.insert(0, _p)
            break
    import concourse.bass as bass  # noqa: F401

import concourse.tile as tile
from concourse import bacc, bass_utils, mybir
from concourse.masks import make_identity

F32 = mybir.dt.float32
BF16 = mybir.dt.bfloat16
I16 = mybir.dt.int16
F8 = mybir.dt.float8e4
AF = mybir.ActivationFunctionType
ALU = mybir.AluOpType
DR = mybir.MatmulPerfMode.DoubleRow

B, N_Q, N_KV, D, H, HD = 2, 2048, 2048, 512, 8, 64
N_CORES = 8
QBLK = N_Q // 4          # 512 query rows per core
DC = D // 128            # 4 partition chunks of the model dim
KC = N_KV // 128         # 16 key chunks
NB = N_KV // 512         # 4 key blocks (DMA/projection granularity)
LN_EPS = 1e-5
SM_SCALE = 1.0 / np.sqrt(HD)

# Schraudolph fast-exp constants (bf16 bit trick): bf16 bits of ~exp(s/8) are
# int16(s * A + Bc). Renormalization cancels the piecewise-linear bias.
SCH_A = float((2.0 ** 7) / np.log(2.0) / 8.0)
SCH_B = float(127.0 * (2 ** 7) - 6.0)
# kc tiles with (kc % 16) < SPLIT go to VectorE fast-exp, rest to ScalarE Exp.
SPLIT = 6


def _build_program(trace=False):
    nc = bacc.Bacc("TRN2", target_bir_lowering=False, debug=False,
                   num_devices=N_CORES)

    def din(name, shape, dt):
        return nc.dram_tensor(name, shape, dt, kind="ExternalInput").ap()

    xqT = din("xqT", [128, DC, QBLK], BF16)   # query block^T, chunk layout
    kvT = din("kvT", [128, DC, N_KV], BF16)   # key_value^T, chunk layout
    wq = din("wq", [128, DC, D], BF16)        # weights, chunk layout
    wk = din("wk", [128, DC, D], BF16)
    kvt8 = din("kvt8", [128, DC, N_KV], F8)   # key_value^T, DR subtile layout
    wv8 = din("wv8", [128, DC, D], F8)        # Wv, DR subtile layout
    xqr = din("xqr", [QBLK, D], F32)          # query block + bv (residual base)
    bqc = din("bqc", [128, DC], F32)          # bq chunked [128, 4]
    gammab = din("gammab", [128, D], F32)
    betab = din("betab", [128, D], F32)
    out = nc.dram_tensor("out", [QBLK, D], F32, kind="ExternalOutput").ap()

    with tile.TileContext(nc) as tc:
        with (
            tc.tile_pool(name="persist", bufs=1) as pp,
            tc.tile_pool(name="work", bufs=2) as wkp,
            tc.tile_pool(name="small", bufs=8) as smp,
            tc.tile_pool(name="scratch_ps", bufs=2, space="PSUM") as sps,
            tc.tile_pool(name="sc_ps", bufs=2, space="PSUM") as scps,
            tc.tile_pool(name="pv_ps", bufs=1, space="PSUM") as pvps,
        ):
            # ---- persistent SBUF tiles ----
            wq_sb = pp.tile([128, DC, D], BF16, name="wq", tag="wq")
            wk_sb = pp.tile([128, DC, D], BF16, name="wk", tag="wk")
            xqt_sb = pp.tile([128, DC, QBLK], BF16, name="xqt", tag="xqt")
            kvt_sb = [pp.tile([128, DC, 512], BF16, name=f"kvt{n}", tag=f"kvt{n}")
                      for n in range(NB)]
            wv_sb = pp.tile([128, DC, D], F8, name="wv8", tag="wv8")
            kvt8_sb = [pp.tile([128, DC, N_KV // 2], F8, name=f"kvt8_{i}",
                               tag=f"kvt8_{i}") for i in range(2)]
            xqr_sb = [pp.tile([128, D], F32, name=f"xqr{q}", tag=f"xqr{q}")
                      for q in range(4)]
            bqc_sb = pp.tile([128, DC], F32, name="bqc", tag="bqc")
            gam_sb = pp.tile([128, D], F32, name="gam", tag="gam")
            bet_sb = pp.tile([128, D], F32, name="bet", tag="bet")

            # ---- loads spread across the three DMA queues, criticals first ----
            # sync: wq, kvt0, kvt2 | scalar: xqt, kvt1, kvt3 | gpsimd: bqc, wk,
            # kvt8 halves, wv8, then LN consts.
            nc.gpsimd.dma_start(bqc_sb[:], bqc)
            nc.sync.dma_start(wq_sb[:], wq)
            nc.scalar.dma_start(xqt_sb[:], xqT)
            nc.gpsimd.dma_start(wk_sb[:], wk)
            nc.sync.dma_start(kvt_sb[0][:], kvT[:, :, 0:512])
            nc.scalar.dma_start(kvt_sb[1][:], kvT[:, :, 512:1024])
            nc.gpsimd.dma_start(kvt8_sb[0][:], kvt8[:, :, 0:N_KV // 2])
            nc.sync.dma_start(kvt_sb[2][:], kvT[:, :, 1024:1536])
            nc.scalar.dma_start(kvt_sb[3][:], kvT[:, :, 1536:2048])
            nc.gpsimd.dma_start(kvt8_sb[1][:], kvt8[:, :, N_KV // 2:])
            nc.gpsimd.dma_start(wv_sb[:], wv8)
            for q in range(4):
                nc.gpsimd.dma_start(xqr_sb[q][:], xqr[q * 128:(q + 1) * 128, :])
            nc.gpsimd.dma_start(gam_sb[:], gammab)
            nc.gpsimd.dma_start(bet_sb[:], betab)
            ident = pp.tile([128, 128], F32, name="ident", tag="ident")
            make_identity(nc, ident[:])
            eps_sb = pp.tile([128, 1], F32, name="eps", tag="eps")
            nc.vector.memset(eps_sb[:], float(LN_EPS))

            qt_sb = [pp.tile([128, QBLK], BF16, name=f"qt{m}", tag=f"qt{m}")
                     for m in range(DC)]
            # kt as per-key-block tiles so scores can start before all of
            # kvT has landed / been projected
            kt_sb = [[pp.tile([128, 512], BF16, name=f"kt{m}_{n}",
                              tag=f"kt{m}_{n}") for n in range(NB)]
                     for m in range(DC)]
            vaug_sb = [pp.tile([128, H * (HD + 1)], BF16, name=f"va{t}", tag=f"va{t}")
                       for t in range(KC)]
            c_sb = [pp.tile([128, D], F32, name=f"csb{q}", tag=f"csb{q}")
                    for q in range(4)]

            def proj_qt(m):
                ps = sps.tile([128, QBLK], F32, name="ps_qt", tag="scratch")
                for c in range(DC):
                    nc.tensor.matmul(
                        ps[:], wq_sb[:, c, m * 128:(m + 1) * 128],
                        xqt_sb[:, c, :], start=(c == 0), stop=(c == DC - 1))
                nc.vector.tensor_scalar(
                    out=qt_sb[m][:], in0=ps[:], scalar1=bqc_sb[:, m:m + 1],
                    scalar2=None, op0=ALU.add)

            def proj_kt_n(n):
                # all four output chunks for key-block n (starts as soon as
                # kvt block n lands)
                for m in range(DC):
                    ps = sps.tile([128, 512], F32, name="ps_kt", tag="scratch")
                    for c in range(DC):
                        nc.tensor.matmul(
                            ps[:], wk_sb[:, c, m * 128:(m + 1) * 128],
                            kvt_sb[n][:, c, :],
                            start=(c == 0), stop=(c == DC - 1))
                    nc.scalar.copy(kt_sb[m][n][:], ps[:])

            def proj_v(t):
                half = kvt8_sb[t // (KC // 2)]
                toff = (t % (KC // 2)) * 128
                ps = sps.tile([128, D], F32, name="ps_v", tag="scratch")
                for cp in range(2):
                    nc.tensor.matmul(
                        ps[:], half[:, 2 * cp:2 * cp + 2, toff:toff + 128],
                        wv_sb[:, 2 * cp:2 * cp + 2, :],
                        start=(cp == 0), stop=(cp == 1), perf_mode=DR)
                va3 = vaug_sb[t][:].rearrange("p (h d) -> p h d", h=H)
                nc.vector.tensor_scalar(
                    out=va3[:, :, 0:HD],
                    in0=ps[:].rearrange("p (h d) -> p h d", h=H),
                    scalar1=0.0, scalar2=None, op0=ALU.add)
                nc.gpsimd.memset(vaug_sb[t][:, HD::HD + 1], 1.0)

            def scores_exp(g, kc):
                psc = scps.tile([128, 2 * QBLK], F32, name="psc", tag="sc")
                n, kb = divmod(kc, NB)
                for j in range(2):
                    nc.tensor.matmul(
                        psc[:, j * QBLK:(j + 1) * QBLK],
                        kt_sb[g][n][j * 64:(j + 1) * 64, kb * 128:(kb + 1) * 128],
                        qt_sb[g][j * 64:(j + 1) * 64, :],
                        start=True, stop=True, tile_position=(j * 64, 0))
                pt = wkp.tile([128, 2 * QBLK], I16, name="pt", tag="pt")
                if (kc % KC) < SPLIT:
                    nc.vector.tensor_scalar(
                        out=pt[:], in0=psc[:], scalar1=float(SCH_A),
                        scalar2=float(SCH_B), op0=ALU.mult, op1=ALU.add)
                else:
                    nc.scalar.activation(pt[:].bitcast(BF16), psc[:],
                                         AF.Exp, scale=float(SM_SCALE))
                return pt

            def pv(g, kc, ppv, pt):
                ptb = pt[:].bitcast(BF16)
                for j in range(2):
                    h = 2 * g + j
                    nc.tensor.matmul(
                        ppv[j][:],
                        vaug_sb[kc][:, h * (HD + 1):(h + 1) * (HD + 1)],
                        ptb[:, j * QBLK:(j + 1) * QBLK],
                        start=(kc == 0), stop=(kc == KC - 1))

            def finish_pair(g, ppv, then_ln=False):
                cts = []
                for j in range(2):
                    ct = wkp.tile([HD + 1, QBLK], F32, name="ct", tag="ct")
                    nc.vector.tensor_copy(ct[:], ppv[j][:])
                    cts.append(ct)
                for q in range(4):
                    # both heads of the pair transpose into one PSUM tile
                    ptr = sps.tile([128, 2 * (HD + 1)], F32, name="ptr",
                                   tag="scratch")
                    for j in range(2):
                        nc.tensor.transpose(
                            ptr[:, j * (HD + 1):(j + 1) * (HD + 1)],
                            cts[j][:, q * 128:(q + 1) * 128],
                            ident[0:HD + 1, 0:HD + 1])
                    linv = smp.tile([128, 2], F32, name="linv", tag="linv")
                    nc.vector.reciprocal(linv[:], ptr[:, HD::HD + 1])
                    for j in range(2):
                        h = 2 * g + j
                        nc.vector.tensor_scalar(
                            out=c_sb[q][:, h * HD:(h + 1) * HD],
                            in0=ptr[:, j * (HD + 1):j * (HD + 1) + HD],
                            scalar1=linv[:, j:j + 1], scalar2=None,
                            op0=ALU.mult)
                    if then_ln:
                        layer_norm(q)

            # ---- residual + LayerNorm ----
            def layer_norm(q):
                resid = wkp.tile([128, D], F32, name="resid", tag="resid")
                rowsum = smp.tile([128, 1], F32, name="rowsum", tag="rowsum")
                nc.vector.scalar_tensor_tensor(
                    out=resid[:], in0=c_sb[q][:], scalar=0.0, in1=xqr_sb[q][:],
                    op0=ALU.bypass, op1=ALU.add, accum_out=rowsum[:])
                sq = wkp.tile([128, D], F32, name="sq", tag="sq")
                sqs = smp.tile([128, 1], F32, name="sqs", tag="sqs")
                nc.scalar.activation(sq[:], resid[:], AF.Square, accum_out=sqs[:])
                mu = smp.tile([128, 1], F32, name="mu", tag="mu")
                nc.vector.tensor_scalar_mul(mu[:], rowsum[:], 1.0 / D)
                musq = smp.tile([128, 1], F32, name="musq", tag="musq")
                nc.vector.tensor_tensor(out=musq[:], in0=mu[:], in1=mu[:], op=ALU.mult)
                var = smp.tile([128, 1], F32, name="var", tag="var")
                nc.vector.scalar_tensor_tensor(
                    out=var[:], in0=sqs[:], scalar=1.0 / D, in1=musq[:],
                    op0=ALU.mult, op1=ALU.subtract)
                std = smp.tile([128, 1], F32, name="std", tag="std")
                nc.scalar.activation(std[:], var[:], AF.Sqrt, bias=eps_sb[:])
                inv = smp.tile([128, 1], F32, name="inv", tag="inv")
                nc.vector.reciprocal(inv[:], std[:])
                xcn = wkp.tile([128, D], F32, name="xcn", tag="xcn")
                nc.vector.tensor_scalar(
                    out=xcn[:], in0=resid[:], scalar1=mu[:], scalar2=inv[:],
                    op0=ALU.subtract, op1=ALU.mult)
                t2 = wkp.tile([128, D], F32, name="t2", tag="t2")
                nc.vector.tensor_tensor(out=t2[:], in0=xcn[:], in1=gam_sb[:],
                                        op=ALU.mult)
                o = wkp.tile([128, D], F32, name="o", tag="o")
                nc.vector.tensor_tensor(out=o[:], in0=t2[:], in1=bet_sb[:],
                                        op=ALU.add)
                nc.sync.dma_start(out[q * 128:(q + 1) * 128, :], o[:])

            # ---- phase schedule ----
            # Phase 0: all projections, pipelined with DMA arrival.
            for m in range(DC):
                proj_qt(m)
            proj_kt_n(0)
            for t in range(0, 4):
                proj_v(t)
            proj_kt_n(1)
            for t in range(4, 8):
                proj_v(t)
            proj_kt_n(2)
            proj_kt_n(3)
            for t in range(8, KC):
                proj_v(t)
            # Phase 1: attention, pure scores/exp/PV; finish of pair g
            # overlaps pair g+1.
            pending = None  # (g, ppv) awaiting finish_pair
            for g in range(DC):
                ppv = [pvps.tile([HD + 1, QBLK], F32, name=f"pv{j}", tag=f"pv{j}")
                       for j in range(2)]
                pt_prev = None
                for kc in range(KC):
                    pt = scores_exp(g, kc)
                    if pt_prev is not None:
                        pv(g, kc - 1, ppv, pt_prev)
                    pt_prev = pt
                pv(g, KC - 1, ppv, pt_prev)
                if pending is not None:
                    finish_pair(*pending)
                pending = (g, ppv)
            finish_pair(*pending, then_ln=True)

    nc.compile()
    return nc


_PROGRAM = None


def _get_program():
    global _PROGRAM
    if _PROGRAM is None:
        _PROGRAM = _build_program()
    return _PROGRAM


def _chunk4(a, dtype):
    """[512, N] -> [128, 4, N] K-subtile/chunk layout."""
    return np.ascontiguousarray(
        np.asarray(a, np.float32).reshape(DC, 128, -1).transpose(1, 0, 2)
    ).astype(dtype)


def _make_in_maps(query, key_value, Wq, bq, Wk, bk, Wv, bv, ln_gamma, ln_beta):
    f32 = np.float32
    f8 = ml_dtypes.float8_e4m3fn
    bf = ml_dtypes.bfloat16
    wq_b = _chunk4(Wq, bf)
    wk_b = _chunk4(Wk, bf)
    wv8 = _chunk4(Wv, f8)
    bqc = np.ascontiguousarray(bq.reshape(DC, 128).T, dtype=f32)
    gammab = np.ascontiguousarray(
        np.broadcast_to(ln_gamma[None, :], (128, D)), dtype=f32)
    betab = np.ascontiguousarray(
        np.broadcast_to(ln_beta[None, :], (128, D)), dtype=f32)
    kvT = [_chunk4(key_value[b].T, bf) for b in range(B)]
    kvt8 = [_chunk4(key_value[b].T, f8) for b in range(B)]
    in_maps = []
    for core in range(N_CORES):
        b, qb = divmod(core, 4)
        blk = query[b, qb * QBLK:(qb + 1) * QBLK, :]
        in_maps.append({
            "xqT": _chunk4(blk.T, bf),
            "xqr": np.ascontiguousarray(blk + bv[None, :], dtype=f32),
            "kvT": kvT[b], "kvt8": kvt8[b],
            "wq": wq_b, "wk": wk_b, "wv8": wv8,
            "bqc": bqc,
            "gammab": gammab, "betab": betab,
        })
    return in_maps


def kernel(query, key_value, Wq, bq, Wk, bk, Wv, bv, ln_gamma, ln_beta,
           _trace=False, _trace_kwargs=None):
    args = [np.asarray(a, dtype=np.float32) for a in
            (query, key_value, Wq, bq, Wk, bk, Wv, bv, ln_gamma, ln_beta)]
    nc = _get_program()
    in_maps = _make_in_maps(*args)
    res = bass_utils.run_bass_kernel_spmd(
        nc, in_maps, core_ids=list(range(N_CORES)), trace=_trace,
        **(_trace_kwargs or {}))
    out = np.empty((B, N_Q, D), np.float32)
    for core in range(N_CORES):
        b, qb = divmod(core, 4)
        out[b, qb * QBLK:(qb + 1) * QBLK, :] = res.results[core]["out"]
    if _trace:
        return out, res
    return out


# revision 22
# speedup vs baseline: 1.0931x; 1.0383x over previous
"""Trainium2 Bass kernel for CrossModalAttention (attention + residual + LayerNorm).

Math: the reference concatenates [query, key_value], projects Q/K/V, attends with an
additive -10000 mask on key columns < N_q, and keeps only the query-row outputs.
With scores ~ N(0,1), exp(score - 10000 - rowmax) underflows to exactly 0 in fp32,
so the computation is exactly standard cross-attention:
    Q = query @ Wq + bq ; K = key_value @ Wk + bk ; V = key_value @ Wv + bv
    C = softmax(Q K^T / 8) V ;  out = LayerNorm(query + C) * gamma + beta

Additional exact simplifications used here:
  - bk drops entirely: (q+bq).(k+bk) differs from (q+bq).k by a per-query constant,
    which softmax cancels.
  - bv folds into the residual: softmax rows sum to 1, so ctx(V+bv) = ctx(V) + bv;
    the host adds bv to the residual input.

Sharding: 8 cores = 2 batches x 4 query-blocks of 512 rows. Each core computes the
K/V projections for its batch (duplicated across the 4 cores of a batch) and full
8-head attention + LN for its query block.

Per-core kernel (v5):
  - All projections run up-front, pipelined with DMA arrival (kvT split in four
    key-blocks, loads spread over four DMA queues); the attention loop then runs
    pure scores/exp/PV at the softmax-exp pace.
  - Q/K projections in bf16; V projection in fp8e4 DoubleRow (2 MACs/cell).
  - scores S^T[k, q] per head via K=64 matmuls, two heads packed with
    tile_position row tiling (bf16 operands).
  - softmax exp split across engines: ScalarE runs true Exp (scale=1/8 folded),
    VectorE runs Schraudolph fast-exp (int16(x*184.665/8 + B) bitcast to bf16,
    ~+-4% relative, cancels in softmax renorm; final output error ~0.7%).
  - PV with ones-augmented V (M=65): row 64 accumulates the softmax denominator.
  - PE-transpose C^T -> C (both heads of a pair into one PSUM tile), normalize
    by 1/l, residual + LayerNorm on Vector/Scalar.
"""

import os
import sys

import numpy as np
import ml_dtypes

try:
    import concourse.bass as bass  # noqa: F401
except ImportError:
    for _p in ("/opt/trn_rl_repo", "/root/.axon_site/_ro/trn_rl_repo"):
        if os.path.isdir(_p):
            sys.path.insert(0, _p)
            break
    import concourse.bass as bass  # noqa: F401

import concourse.tile as tile
from concourse import bacc, bass_utils, mybir
from concourse.masks import make_identity

F32 = mybir.dt.float32
BF16 = mybir.dt.bfloat16
I16 = mybir.dt.int16
F8 = mybir.dt.float8e4
AF = mybir.ActivationFunctionType
ALU = mybir.AluOpType
DR = mybir.MatmulPerfMode.DoubleRow

B, N_Q, N_KV, D, H, HD = 2, 2048, 2048, 512, 8, 64
N_CORES = 8
QBLK = N_Q // 4          # 512 query rows per core
DC = D // 128            # 4 partition chunks of the model dim
KC = N_KV // 128         # 16 key chunks
NB = N_KV // 512         # 4 key blocks (DMA/projection granularity)
LN_EPS = 1e-5
SM_SCALE = 1.0 / np.sqrt(HD)

# Schraudolph fast-exp constants (bf16 bit trick): bf16 bits of ~exp(s/8) are
# int16(s * A + Bc). Renormalization cancels the piecewise-linear bias.
SCH_A = float((2.0 ** 7) / np.log(2.0) / 8.0)
SCH_B = float(127.0 * (2 ** 7) - 6.0)
# kc tiles with (kc % 16) < SPLIT go to VectorE fast-exp, rest to ScalarE Exp.
SPLIT = 6


def _build_program(trace=False):
    nc = bacc.Bacc("TRN2", target_bir_lowering=False, debug=False,
                   num_devices=N_CORES)

    def din(name, shape, dt):
        return nc.dram_tensor(name, shape, dt, kind="ExternalInput").ap()

    xqT = din("xqT", [128, DC, QBLK], BF16)   # query block^T, chunk layout
    kvT = din("kvT", [128, DC, N_KV], BF16)   # key_value^T, chunk layout
    wq = din("wq", [128, DC, D], BF16)        # weights, chunk layout
    wk = din("wk", [128, DC, D], BF16)
    kvt8 = din("kvt8", [128, DC, N_KV], F8)   # key_value^T, DR subtile layout
    wv8 = din("wv8", [128, DC, D], F8)        # Wv, DR subtile layout
    xqr = din("xqr", [QBLK, D], F32)          # query block + bv (residual base)
    bqc = din("bqc", [128, DC], F32)          # bq chunked [128, 4]
    gammab = din("gammab", [128, D], F32)
    betab = din("betab", [128, D], F32)
    out = nc.dram_tensor("out", [QBLK, D], F32, kind="ExternalOutput").ap()

    with tile.TileContext(nc) as tc:
        with (
            tc.tile_pool(name="persist", bufs=1) as pp,
            tc.tile_pool(name="work", bufs=2) as wkp,
            tc.tile_pool(name="ptpool", bufs=4) as ptp,
            tc.tile_pool(name="small", bufs=8) as smp,
            tc.tile_pool(name="scratch_ps", bufs=2, space="PSUM") as sps,
            tc.tile_pool(name="sc_ps", bufs=2, space="PSUM") as scps,
            tc.tile_pool(name="pv_ps", bufs=1, space="PSUM") as pvps,
        ):
            # ---- persistent SBUF tiles ----
            wq_sb = pp.tile([128, DC, D], BF16, name="wq", tag="wq")
            wk_sb = pp.tile([128, DC, D], BF16, name="wk", tag="wk")
            xqt_sb = pp.tile([128, DC, QBLK], BF16, name="xqt", tag="xqt")
            kvt_sb = [pp.tile([128, DC, 512], BF16, name=f"kvt{n}", tag=f"kvt{n}")
                      for n in range(NB)]
            wv_sb = pp.tile([128, DC, D], F8, name="wv8", tag="wv8")
            kvt8_sb = [pp.tile([128, DC, 512], F8, name=f"kvt8_{i}",
                               tag=f"kvt8_{i}") for i in range(4)]
            xqr_sb = [pp.tile([128, D], F32, name=f"xqr{q}", tag=f"xqr{q}")
                      for q in range(4)]
            bqc_sb = pp.tile([128, DC], F32, name="bqc", tag="bqc")
            gam_sb = pp.tile([128, D], F32, name="gam", tag="gam")
            bet_sb = pp.tile([128, D], F32, name="bet", tag="bet")

            # ---- loads spread across the three DMA queues, ordered by the
            # time the attention pipeline will need each block ----
            nc.gpsimd.dma_start(bqc_sb[:], bqc)
            nc.sync.dma_start(wq_sb[:], wq)
            nc.scalar.dma_start(xqt_sb[:], xqT)
            nc.gpsimd.dma_start(wk_sb[:], wk)
            nc.gpsimd.dma_start(wv_sb[:], wv8)
            nc.gpsimd.dma_start(kvt8_sb[0][:], kvt8[:, :, 0:512])
            nc.sync.dma_start(kvt_sb[0][:], kvT[:, :, 0:512])
            nc.scalar.dma_start(kvt_sb[1][:], kvT[:, :, 512:1024])
            nc.sync.dma_start(kvt8_sb[1][:], kvt8[:, :, 512:1024])
            nc.scalar.dma_start(kvt8_sb[2][:], kvt8[:, :, 1024:1536])
            nc.gpsimd.dma_start(kvt8_sb[3][:], kvt8[:, :, 1536:2048])
            nc.sync.dma_start(kvt_sb[2][:], kvT[:, :, 1024:1536])
            nc.scalar.dma_start(kvt_sb[3][:], kvT[:, :, 1536:2048])
            for q in range(4):
                nc.gpsimd.dma_start(xqr_sb[q][:], xqr[q * 128:(q + 1) * 128, :])
            nc.gpsimd.dma_start(gam_sb[:], gammab)
            nc.gpsimd.dma_start(bet_sb[:], betab)
            ident = pp.tile([128, 128], F32, name="ident", tag="ident")
            make_identity(nc, ident[:])
            eps_sb = pp.tile([128, 1], F32, name="eps", tag="eps")
            nc.vector.memset(eps_sb[:], float(LN_EPS))

            qt_sb = [pp.tile([128, QBLK], BF16, name=f"qt{m}", tag=f"qt{m}")
                     for m in range(DC)]
            # kt as per-key-block tiles so scores can start before all of
            # kvT has landed / been projected
            kt_sb = [[pp.tile([128, 512], BF16, name=f"kt{m}_{n}",
                              tag=f"kt{m}_{n}") for n in range(NB)]
                     for m in range(DC)]
            vaug_sb = [pp.tile([128, H * (HD + 1)], BF16, name=f"va{t}", tag=f"va{t}")
                       for t in range(KC)]
            c_sb = [pp.tile([128, D], F32, name=f"csb{q}", tag=f"csb{q}")
                    for q in range(4)]

            def proj_qt(m):
                ps = sps.tile([128, QBLK], F32, name="ps_qt", tag="scratch")
                for c in range(DC):
                    nc.tensor.matmul(
                        ps[:], wq_sb[:, c, m * 128:(m + 1) * 128],
                        xqt_sb[:, c, :], start=(c == 0), stop=(c == DC - 1))
                nc.vector.tensor_scalar(
                    out=qt_sb[m][:], in0=ps[:], scalar1=bqc_sb[:, m:m + 1],
                    scalar2=None, op0=ALU.add)

            def proj_kt_n(n):
                # all four output chunks for key-block n (starts as soon as
                # kvt block n lands)
                for m in range(DC):
                    ps = sps.tile([128, 512], F32, name="ps_kt", tag="scratch")
                    for c in range(DC):
                        nc.tensor.matmul(
                            ps[:], wk_sb[:, c, m * 128:(m + 1) * 128],
                            kvt_sb[n][:, c, :],
                            start=(c == 0), stop=(c == DC - 1))
                    nc.scalar.copy(kt_sb[m][n][:], ps[:])

            def proj_v(t):
                quarter = kvt8_sb[t // 4]
                toff = (t % 4) * 128
                ps = sps.tile([128, D], F32, name="ps_v", tag="scratch")
                for cp in range(2):
                    nc.tensor.matmul(
                        ps[:], quarter[:, 2 * cp:2 * cp + 2, toff:toff + 128],
                        wv_sb[:, 2 * cp:2 * cp + 2, :],
                        start=(cp == 0), stop=(cp == 1), perf_mode=DR)
                va3 = vaug_sb[t][:].rearrange("p (h d) -> p h d", h=H)
                nc.vector.tensor_scalar(
                    out=va3[:, :, 0:HD],
                    in0=ps[:].rearrange("p (h d) -> p h d", h=H),
                    scalar1=0.0, scalar2=None, op0=ALU.add)
                nc.gpsimd.memset(vaug_sb[t][:, HD::HD + 1], 1.0)

            def scores_exp(g, kc):
                psc = scps.tile([128, 2 * QBLK], F32, name="psc", tag="sc")
                n, kb = divmod(kc, NB)
                for j in range(2):
                    nc.tensor.matmul(
                        psc[:, j * QBLK:(j + 1) * QBLK],
                        kt_sb[g][n][j * 64:(j + 1) * 64, kb * 128:(kb + 1) * 128],
                        qt_sb[g][j * 64:(j + 1) * 64, :],
                        start=True, stop=True, tile_position=(j * 64, 0))
                pt = ptp.tile([128, 2 * QBLK], I16, name="pt", tag="pt")
                if (kc % KC) < SPLIT:
                    nc.vector.tensor_scalar(
                        out=pt[:], in0=psc[:], scalar1=float(SCH_A),
                        scalar2=float(SCH_B), op0=ALU.mult, op1=ALU.add)
                else:
                    nc.scalar.activation(pt[:].bitcast(BF16), psc[:],
                                         AF.Exp, scale=float(SM_SCALE))
                return pt

            def pv(g, kc, ppv, pt):
                ptb = pt[:].bitcast(BF16)
                for j in range(2):
                    h = 2 * g + j
                    nc.tensor.matmul(
                        ppv[j][:],
                        vaug_sb[kc][:, h * (HD + 1):(h + 1) * (HD + 1)],
                        ptb[:, j * QBLK:(j + 1) * QBLK],
                        start=(kc == 0), stop=(kc == KC - 1))

            def finish_pair(g, ppv, then_ln=False):
                cts = []
                for j in range(2):
                    ct = wkp.tile([HD + 1, QBLK], F32, name="ct", tag="ct")
                    nc.vector.tensor_copy(ct[:], ppv[j][:])
                    cts.append(ct)
                for q in range(4):
                    # both heads of the pair transpose into one PSUM tile
                    ptr = sps.tile([128, 2 * (HD + 1)], F32, name="ptr",
                                   tag="scratch")
                    for j in range(2):
                        nc.tensor.transpose(
                            ptr[:, j * (HD + 1):(j + 1) * (HD + 1)],
                            cts[j][:, q * 128:(q + 1) * 128],
                            ident[0:HD + 1, 0:HD + 1])
                    linv = smp.tile([128, 2], F32, name="linv", tag="linv")
                    nc.vector.reciprocal(linv[:], ptr[:, HD::HD + 1])
                    for j in range(2):
                        h = 2 * g + j
                        nc.vector.tensor_scalar(
                            out=c_sb[q][:, h * HD:(h + 1) * HD],
                            in0=ptr[:, j * (HD + 1):j * (HD + 1) + HD],
                            scalar1=linv[:, j:j + 1], scalar2=None,
                            op0=ALU.mult)
                    if then_ln:
                        layer_norm(q)

            # ---- residual + LayerNorm ----
            def layer_norm(q):
                resid = wkp.tile([128, D], F32, name="resid", tag="resid")
                rowsum = smp.tile([128, 1], F32, name="rowsum", tag="rowsum")
                nc.vector.scalar_tensor_tensor(
                    out=resid[:], in0=c_sb[q][:], scalar=0.0, in1=xqr_sb[q][:],
                    op0=ALU.bypass, op1=ALU.add, accum_out=rowsum[:])
                sq = wkp.tile([128, D], F32, name="sq", tag="sq")
                sqs = smp.tile([128, 1], F32, name="sqs", tag="sqs")
                nc.scalar.activation(sq[:], resid[:], AF.Square, accum_out=sqs[:])
                mu = smp.tile([128, 1], F32, name="mu", tag="mu")
                nc.vector.tensor_scalar_mul(mu[:], rowsum[:], 1.0 / D)
                musq = smp.tile([128, 1], F32, name="musq", tag="musq")
                nc.vector.tensor_tensor(out=musq[:], in0=mu[:], in1=mu[:], op=ALU.mult)
                var = smp.tile([128, 1], F32, name="var", tag="var")
                nc.vector.scalar_tensor_tensor(
                    out=var[:], in0=sqs[:], scalar=1.0 / D, in1=musq[:],
                    op0=ALU.mult, op1=ALU.subtract)
                std = smp.tile([128, 1], F32, name="std", tag="std")
                nc.scalar.activation(std[:], var[:], AF.Sqrt, bias=eps_sb[:])
                inv = smp.tile([128, 1], F32, name="inv", tag="inv")
                nc.vector.reciprocal(inv[:], std[:])
                # o = ((resid - mu) * gam) * inv + bet, two fused DVE ops
                t2 = wkp.tile([128, D], F32, name="t2", tag="t2")
                nc.vector.scalar_tensor_tensor(
                    out=t2[:], in0=resid[:], scalar=mu[:], in1=gam_sb[:],
                    op0=ALU.subtract, op1=ALU.mult)
                o = wkp.tile([128, D], F32, name="o", tag="o")
                nc.vector.scalar_tensor_tensor(
                    out=o[:], in0=t2[:], scalar=inv[:], in1=bet_sb[:],
                    op0=ALU.mult, op1=ALU.add)
                nc.sync.dma_start(out[q * 128:(q + 1) * 128, :], o[:])

            # ---- phase schedule ----
            # Minimal prologue: Q projections, key-block 0 of K, first V
            # quarter; all remaining projections are injected between early
            # attention steps so the tensor queue never blocks on late DMAs.
            proj_qt(0)
            proj_kt_n(0)
            for m in range(1, DC):
                proj_qt(m)
            for t in range(0, 4):
                proj_v(t)
            # proj work due at attention step (g, kc), keyed by g * KC + kc
            due = {
                1: [lambda: proj_kt_n(1)],
                2: [lambda: proj_v(4), lambda: proj_v(5)],
                3: [lambda: proj_v(6), lambda: proj_v(7)],
                4: [lambda: proj_kt_n(2)],
                5: [lambda: proj_v(8), lambda: proj_v(9)],
                6: [lambda: proj_v(10), lambda: proj_v(11)],
                7: [lambda: proj_kt_n(3)],
                8: [lambda: proj_v(12), lambda: proj_v(13)],
                9: [lambda: proj_v(14), lambda: proj_v(15)],
            }
            pending = None  # (g, ppv) awaiting finish_pair
            for g in range(DC):
                ppv = [pvps.tile([HD + 1, QBLK], F32, name=f"pv{j}", tag=f"pv{j}")
                       for j in range(2)]
                pt_prev = None
                for kc in range(KC):
                    pt = scores_exp(g, kc)
                    if pt_prev is not None:
                        pv(g, kc - 1, ppv, pt_prev)
                    pt_prev = pt
                    for fn in due.pop(g * KC + kc, ()):
                        fn()
                pv(g, KC - 1, ppv, pt_prev)
                if pending is not None:
                    finish_pair(*pending)
                pending = (g, ppv)
            finish_pair(*pending, then_ln=True)

    nc.compile()
    return nc


_PROGRAM = None


def _get_program():
    global _PROGRAM
    if _PROGRAM is None:
        _PROGRAM = _build_program()
    return _PROGRAM


def _chunk4(a, dtype):
    """[512, N] -> [128, 4, N] K-subtile/chunk layout."""
    return np.ascontiguousarray(
        np.asarray(a, np.float32).reshape(DC, 128, -1).transpose(1, 0, 2)
    ).astype(dtype)


def _make_in_maps(query, key_value, Wq, bq, Wk, bk, Wv, bv, ln_gamma, ln_beta):
    f32 = np.float32
    f8 = ml_dtypes.float8_e4m3fn
    bf = ml_dtypes.bfloat16
    wq_b = _chunk4(Wq, bf)
    wk_b = _chunk4(Wk, bf)
    wv8 = _chunk4(Wv, f8)
    bqc = np.ascontiguousarray(bq.reshape(DC, 128).T, dtype=f32)
    gammab = np.ascontiguousarray(
        np.broadcast_to(ln_gamma[None, :], (128, D)), dtype=f32)
    betab = np.ascontiguousarray(
        np.broadcast_to(ln_beta[None, :], (128, D)), dtype=f32)
    kvT = [_chunk4(key_value[b].T, bf) for b in range(B)]
    kvt8 = [_chunk4(key_value[b].T, f8) for b in range(B)]
    in_maps = []
    for core in range(N_CORES):
        b, qb = divmod(core, 4)
        blk = query[b, qb * QBLK:(qb + 1) * QBLK, :]
        in_maps.append({
            "xqT": _chunk4(blk.T, bf),
            "xqr": np.ascontiguousarray(blk + bv[None, :], dtype=f32),
            "kvT": kvT[b], "kvt8": kvt8[b],
            "wq": wq_b, "wk": wk_b, "wv8": wv8,
            "bqc": bqc,
            "gammab": gammab, "betab": betab,
        })
    return in_maps


def kernel(query, key_value, Wq, bq, Wk, bk, Wv, bv, ln_gamma, ln_beta,
           _trace=False, _trace_kwargs=None):
    args = [np.asarray(a, dtype=np.float32) for a in
            (query, key_value, Wq, bq, Wk, bk, Wv, bv, ln_gamma, ln_beta)]
    nc = _get_program()
    in_maps = _make_in_maps(*args)
    res = bass_utils.run_bass_kernel_spmd(
        nc, in_maps, core_ids=list(range(N_CORES)), trace=_trace,
        **(_trace_kwargs or {}))
    out = np.empty((B, N_Q, D), np.float32)
    for core in range(N_CORES):
        b, qb = divmod(core, 4)
        out[b, qb * QBLK:(qb + 1) * QBLK, :] = res.results[core]["out"]
    if _trace:
        return out, res
    return out


# revision 28
# speedup vs baseline: 1.1522x; 1.0540x over previous
"""Trainium2 Bass kernel for CrossModalAttention (attention + residual + LayerNorm).

Math: the reference concatenates [query, key_value], projects Q/K/V, attends with an
additive -10000 mask on key columns < N_q, and keeps only the query-row outputs.
With scores ~ N(0,1), exp(score - 10000 - rowmax) underflows to exactly 0 in fp32,
so the computation is exactly standard cross-attention:
    Q = query @ Wq + bq ; K = key_value @ Wk + bk ; V = key_value @ Wv + bv
    C = softmax(Q K^T / 8) V ;  out = LayerNorm(query + C) * gamma + beta

Additional exact simplifications used here:
  - bk drops entirely: (q+bq).(k+bk) differs from (q+bq).k by a per-query constant,
    which softmax cancels.
  - bv folds into the residual: softmax rows sum to 1, so ctx(V+bv) = ctx(V) + bv;
    the host adds bv to the residual input.

Sharding: 8 cores = 2 batches x 4 query-blocks of 512 rows. Each core computes the
K/V projections for its batch (duplicated across the 4 cores of a batch) and full
8-head attention + LN for its query block.

Per-core kernel (v5):
  - All projections run up-front, pipelined with DMA arrival (kvT split in four
    key-blocks, loads spread over four DMA queues); the attention loop then runs
    pure scores/exp/PV at the softmax-exp pace.
  - Q/K projections in bf16; V projection in fp8e4 DoubleRow (2 MACs/cell).
  - scores S^T[k, q] per head via K=64 matmuls, two heads packed with
    tile_position row tiling (bf16 operands).
  - softmax exp split across engines: ScalarE runs true Exp (scale=1/8 folded),
    VectorE runs Schraudolph fast-exp (int16(x*184.665/8 + B) bitcast to bf16,
    ~+-4% relative, cancels in softmax renorm; final output error ~0.7%).
  - PV with ones-augmented V (M=65): row 64 accumulates the softmax denominator.
  - PE-transpose C^T -> C (both heads of a pair into one PSUM tile), normalize
    by 1/l, residual + LayerNorm on Vector/Scalar.
"""

import os
import sys

import numpy as np
import ml_dtypes

try:
    import concourse.bass as bass  # noqa: F401
except ImportError:
    for _p in ("/opt/trn_rl_repo", "/root/.axon_site/_ro/trn_rl_repo"):
        if os.path.isdir(_p):
            sys.path.insert(0, _p)
            break
    import concourse.bass as bass  # noqa: F401

import concourse.tile as tile
from concourse import bacc, bass_utils, mybir
from concourse.masks import make_identity

F32 = mybir.dt.float32
BF16 = mybir.dt.bfloat16
I16 = mybir.dt.int16
F8 = mybir.dt.float8e4
AF = mybir.ActivationFunctionType
ALU = mybir.AluOpType
DR = mybir.MatmulPerfMode.DoubleRow

B, N_Q, N_KV, D, H, HD = 2, 2048, 2048, 512, 8, 64
N_CORES = 8
QBLK = N_Q // 4          # 512 query rows per core
DC = D // 128            # 4 partition chunks of the model dim
KC = N_KV // 128         # 16 key chunks
NB = N_KV // 512         # 4 key blocks (DMA/projection granularity)
LN_EPS = 1e-5
SM_SCALE = 1.0 / np.sqrt(HD)

# Schraudolph fast-exp constants (bf16 bit trick): bf16 bits of ~exp(s/8) are
# int16(s * A + Bc). Renormalization cancels the piecewise-linear bias.
SCH_A = float((2.0 ** 7) / np.log(2.0) / 8.0)
SCH_B = float(127.0 * (2 ** 7) - 6.0)
# kc tiles with (kc % 16) < SPLIT go to VectorE fast-exp, rest to ScalarE Exp.
SPLIT = 6


def _build_program(trace=False):
    nc = bacc.Bacc("TRN2", target_bir_lowering=False, debug=False,
                   num_devices=N_CORES)

    def din(name, shape, dt):
        return nc.dram_tensor(name, shape, dt, kind="ExternalInput").ap()

    xqT = din("xqT", [128, DC, QBLK], BF16)   # query block^T, chunk layout
    kvT = din("kvT", [128, DC, N_KV], BF16)   # key_value^T, chunk layout
    wq = din("wq", [128, DC, D], BF16)        # weights, chunk layout
    wk = din("wk", [128, DC, D], BF16)
    kvt8 = din("kvt8", [128, DC, N_KV], F8)   # key_value^T, DR subtile layout
    wv8 = din("wv8", [128, DC, D], F8)        # Wv, DR subtile layout
    xqr = din("xqr", [QBLK, D], F32)          # query block + bv (residual base)
    bqc = din("bqc", [128, DC], F32)          # bq chunked [128, 4]
    gammab = din("gammab", [128, D], F32)
    betab = din("betab", [128, D], F32)
    out = nc.dram_tensor("out", [QBLK, D], F32, kind="ExternalOutput").ap()

    with tile.TileContext(nc) as tc:
        with (
            tc.tile_pool(name="persist", bufs=1) as pp,
            tc.tile_pool(name="work", bufs=2) as wkp,
            tc.tile_pool(name="ptpool", bufs=4) as ptp,
            tc.tile_pool(name="small", bufs=8) as smp,
            tc.tile_pool(name="scratch_ps", bufs=2, space="PSUM") as sps,
            tc.tile_pool(name="sc_ps", bufs=2, space="PSUM") as scps,
            tc.tile_pool(name="pv_ps", bufs=1, space="PSUM") as pvps,
        ):
            # ---- persistent SBUF tiles ----
            wq_sb = pp.tile([128, DC, D], BF16, name="wq", tag="wq")
            wk_sb = pp.tile([128, DC, D], BF16, name="wk", tag="wk")
            xqt_sb = pp.tile([128, DC, QBLK], BF16, name="xqt", tag="xqt")
            kvt_sb = [pp.tile([128, DC, 512], BF16, name=f"kvt{n}", tag=f"kvt{n}")
                      for n in range(NB)]
            wv_sb = pp.tile([128, DC, D], F8, name="wv8", tag="wv8")
            kvt8_sb = [pp.tile([128, DC, 512], F8, name=f"kvt8_{i}",
                               tag=f"kvt8_{i}") for i in range(4)]
            xqr_sb = [pp.tile([128, D], F32, name=f"xqr{q}", tag=f"xqr{q}")
                      for q in range(4)]
            bqc_sb = pp.tile([128, DC], F32, name="bqc", tag="bqc")
            gam_sb = pp.tile([128, D], F32, name="gam", tag="gam")
            bet_sb = pp.tile([128, D], F32, name="bet", tag="bet")

            # ---- loads spread across the three DMA queues, ordered by the
            # time the attention pipeline will need each block ----
            nc.gpsimd.dma_start(bqc_sb[:], bqc)
            nc.sync.dma_start(wq_sb[:], wq)
            nc.scalar.dma_start(xqt_sb[:], xqT)
            nc.gpsimd.dma_start(wk_sb[:], wk)
            nc.gpsimd.dma_start(wv_sb[:], wv8)
            nc.gpsimd.dma_start(kvt8_sb[0][:], kvt8[:, :, 0:512])
            nc.sync.dma_start(kvt_sb[0][:], kvT[:, :, 0:512])
            nc.scalar.dma_start(kvt_sb[1][:], kvT[:, :, 512:1024])
            nc.sync.dma_start(kvt8_sb[1][:], kvt8[:, :, 512:1024])
            nc.scalar.dma_start(kvt8_sb[2][:], kvt8[:, :, 1024:1536])
            nc.gpsimd.dma_start(kvt8_sb[3][:], kvt8[:, :, 1536:2048])
            nc.sync.dma_start(kvt_sb[2][:], kvT[:, :, 1024:1536])
            nc.scalar.dma_start(kvt_sb[3][:], kvT[:, :, 1536:2048])
            for q in range(4):
                nc.gpsimd.dma_start(xqr_sb[q][:], xqr[q * 128:(q + 1) * 128, :])
            nc.gpsimd.dma_start(gam_sb[:], gammab)
            nc.gpsimd.dma_start(bet_sb[:], betab)
            ident = pp.tile([128, 128], F32, name="ident", tag="ident")
            make_identity(nc, ident[:])
            eps_sb = pp.tile([128, 1], F32, name="eps", tag="eps")
            nc.vector.memset(eps_sb[:], float(LN_EPS))

            qt_sb = [pp.tile([128, QBLK], BF16, name=f"qt{m}", tag=f"qt{m}")
                     for m in range(DC)]
            # kt as per-key-block tiles so scores can start before all of
            # kvT has landed / been projected
            kt_sb = [[pp.tile([128, 512], BF16, name=f"kt{m}_{n}",
                              tag=f"kt{m}_{n}") for n in range(NB)]
                     for m in range(DC)]
            vaug_sb = [pp.tile([128, H * (HD + 1)], BF16, name=f"va{t}", tag=f"va{t}")
                       for t in range(KC)]
            c_sb = [pp.tile([128, D], F32, name=f"csb{q}", tag=f"csb{q}")
                    for q in range(4)]

            def proj_qt(m):
                ps = sps.tile([128, QBLK], F32, name="ps_qt", tag="scratch")
                for c in range(DC):
                    nc.tensor.matmul(
                        ps[:], wq_sb[:, c, m * 128:(m + 1) * 128],
                        xqt_sb[:, c, :], start=(c == 0), stop=(c == DC - 1))
                nc.vector.tensor_scalar(
                    out=qt_sb[m][:], in0=ps[:], scalar1=bqc_sb[:, m:m + 1],
                    scalar2=None, op0=ALU.add)

            def proj_kt_n(n):
                # all four output chunks for key-block n (starts as soon as
                # kvt block n lands)
                for m in range(DC):
                    ps = sps.tile([128, 512], F32, name="ps_kt", tag="scratch")
                    for c in range(DC):
                        nc.tensor.matmul(
                            ps[:], wk_sb[:, c, m * 128:(m + 1) * 128],
                            kvt_sb[n][:, c, :],
                            start=(c == 0), stop=(c == DC - 1))
                    nc.scalar.copy(kt_sb[m][n][:], ps[:])

            def proj_v(t):
                quarter = kvt8_sb[t // 4]
                toff = (t % 4) * 128
                ps = sps.tile([128, D], F32, name="ps_v", tag="scratch")
                for cp in range(2):
                    nc.tensor.matmul(
                        ps[:], quarter[:, 2 * cp:2 * cp + 2, toff:toff + 128],
                        wv_sb[:, 2 * cp:2 * cp + 2, :],
                        start=(cp == 0), stop=(cp == 1), perf_mode=DR)
                va3 = vaug_sb[t][:].rearrange("p (h d) -> p h d", h=H)
                nc.vector.tensor_scalar(
                    out=va3[:, :, 0:HD],
                    in0=ps[:].rearrange("p (h d) -> p h d", h=H),
                    scalar1=0.0, scalar2=None, op0=ALU.add)
                nc.gpsimd.memset(vaug_sb[t][:, HD::HD + 1], 1.0)

            def scores_exp(g, kc):
                psc = scps.tile([128, 2 * QBLK], F32, name="psc", tag="sc")
                n, kb = divmod(kc, NB)
                for j in range(2):
                    nc.tensor.matmul(
                        psc[:, j * QBLK:(j + 1) * QBLK],
                        kt_sb[g][n][j * 64:(j + 1) * 64, kb * 128:(kb + 1) * 128],
                        qt_sb[g][j * 64:(j + 1) * 64, :],
                        start=True, stop=True, tile_position=(j * 64, 0))
                pt = ptp.tile([128, 2 * QBLK], I16, name="pt", tag="pt")
                # late kc tiles on VectorE: fast-exp runs clear of the early
                # vaug/kt copy window, and pair-finish overlaps ScalarE exps
                if (kc % KC) >= KC - SPLIT:
                    nc.vector.tensor_scalar(
                        out=pt[:], in0=psc[:], scalar1=float(SCH_A),
                        scalar2=float(SCH_B), op0=ALU.mult, op1=ALU.add)
                else:
                    nc.scalar.activation(pt[:].bitcast(BF16), psc[:],
                                         AF.Exp, scale=float(SM_SCALE))
                return pt

            def pv(g, kc, ppv, pt):
                ptb = pt[:].bitcast(BF16)
                for j in range(2):
                    h = 2 * g + j
                    nc.tensor.matmul(
                        ppv[j][:],
                        vaug_sb[kc][:, h * (HD + 1):(h + 1) * (HD + 1)],
                        ptb[:, j * QBLK:(j + 1) * QBLK],
                        start=(kc == 0), stop=(kc == KC - 1))

            def finish_pair(g, ppv, then_ln=False):
                cts = []
                for j in range(2):
                    ct = wkp.tile([HD + 1, QBLK], F32, name="ct", tag="ct")
                    nc.vector.tensor_copy(ct[:], ppv[j][:])
                    cts.append(ct)
                for q in range(4):
                    # both heads of the pair transpose into one PSUM tile
                    ptr = sps.tile([128, 2 * (HD + 1)], F32, name="ptr",
                                   tag="scratch")
                    for j in range(2):
                        nc.tensor.transpose(
                            ptr[:, j * (HD + 1):(j + 1) * (HD + 1)],
                            cts[j][:, q * 128:(q + 1) * 128],
                            ident[0:HD + 1, 0:HD + 1])
                    linv = smp.tile([128, 2], F32, name="linv", tag="linv")
                    nc.vector.reciprocal(linv[:], ptr[:, HD::HD + 1])
                    for j in range(2):
                        h = 2 * g + j
                        nc.vector.tensor_scalar(
                            out=c_sb[q][:, h * HD:(h + 1) * HD],
                            in0=ptr[:, j * (HD + 1):j * (HD + 1) + HD],
                            scalar1=linv[:, j:j + 1], scalar2=None,
                            op0=ALU.mult)
                    if then_ln:
                        layer_norm(q)

            # ---- residual + LayerNorm ----
            def layer_norm(q):
                resid = wkp.tile([128, D], F32, name="resid", tag="resid")
                rowsum = smp.tile([128, 1], F32, name="rowsum", tag="rowsum")
                nc.vector.scalar_tensor_tensor(
                    out=resid[:], in0=c_sb[q][:], scalar=0.0, in1=xqr_sb[q][:],
                    op0=ALU.bypass, op1=ALU.add, accum_out=rowsum[:])
                sq = wkp.tile([128, D], F32, name="sq", tag="sq")
                sqs = smp.tile([128, 1], F32, name="sqs", tag="sqs")
                nc.scalar.activation(sq[:], resid[:], AF.Square, accum_out=sqs[:])
                mu = smp.tile([128, 1], F32, name="mu", tag="mu")
                nc.vector.tensor_scalar_mul(mu[:], rowsum[:], 1.0 / D)
                musq = smp.tile([128, 1], F32, name="musq", tag="musq")
                nc.vector.tensor_tensor(out=musq[:], in0=mu[:], in1=mu[:], op=ALU.mult)
                var = smp.tile([128, 1], F32, name="var", tag="var")
                nc.vector.scalar_tensor_tensor(
                    out=var[:], in0=sqs[:], scalar=1.0 / D, in1=musq[:],
                    op0=ALU.mult, op1=ALU.subtract)
                std = smp.tile([128, 1], F32, name="std", tag="std")
                nc.scalar.activation(std[:], var[:], AF.Sqrt, bias=eps_sb[:])
                inv = smp.tile([128, 1], F32, name="inv", tag="inv")
                nc.vector.reciprocal(inv[:], std[:])
                # o = ((resid - mu) * gam) * inv + bet, two fused DVE ops
                t2 = wkp.tile([128, D], F32, name="t2", tag="t2")
                nc.vector.scalar_tensor_tensor(
                    out=t2[:], in0=resid[:], scalar=mu[:], in1=gam_sb[:],
                    op0=ALU.subtract, op1=ALU.mult)
                o = wkp.tile([128, D], F32, name="o", tag="o")
                nc.vector.scalar_tensor_tensor(
                    out=o[:], in0=t2[:], scalar=inv[:], in1=bet_sb[:],
                    op0=ALU.mult, op1=ALU.add)
                nc.sync.dma_start(out[q * 128:(q + 1) * 128, :], o[:])

            # ---- phase schedule ----
            # Minimal prologue: Q projections, key-block 0 of K, first V
            # quarter; all remaining projections are injected between early
            # attention steps so the tensor queue never blocks on late DMAs.
            proj_qt(0)
            proj_kt_n(0)
            for m in range(1, DC):
                proj_qt(m)
            for t in range(0, 4):
                proj_v(t)
            # proj work due at attention step (g, kc), keyed by g * KC + kc
            due = {
                1: [lambda: proj_kt_n(1)],
                2: [lambda: proj_v(4), lambda: proj_v(5)],
                3: [lambda: proj_v(6), lambda: proj_v(7)],
                4: [lambda: proj_kt_n(2)],
                5: [lambda: proj_v(8), lambda: proj_v(9)],
                6: [lambda: proj_v(10), lambda: proj_v(11)],
                7: [lambda: proj_kt_n(3)],
                8: [lambda: proj_v(12), lambda: proj_v(13)],
                9: [lambda: proj_v(14), lambda: proj_v(15)],
            }
            pending = None  # (g, ppv) awaiting finish_pair
            for g in range(DC):
                ppv = [pvps.tile([HD + 1, QBLK], F32, name=f"pv{j}", tag=f"pv{j}")
                       for j in range(2)]
                pt_prev = None
                for kc in range(KC):
                    pt = scores_exp(g, kc)
                    if pt_prev is not None:
                        pv(g, kc - 1, ppv, pt_prev)
                    pt_prev = pt
                    for fn in due.pop(g * KC + kc, ()):
                        fn()
                pv(g, KC - 1, ppv, pt_prev)
                if pending is not None:
                    finish_pair(*pending)
                pending = (g, ppv)
            finish_pair(*pending, then_ln=True)

    nc.compile()
    return nc


_PROGRAM = None


def _get_program():
    global _PROGRAM
    if _PROGRAM is None:
        _PROGRAM = _build_program()
    return _PROGRAM


def _chunk4(a, dtype):
    """[512, N] -> [128, 4, N] K-subtile/chunk layout."""
    return np.ascontiguousarray(
        np.asarray(a, np.float32).reshape(DC, 128, -1).transpose(1, 0, 2)
    ).astype(dtype)


def _make_in_maps(query, key_value, Wq, bq, Wk, bk, Wv, bv, ln_gamma, ln_beta):
    f32 = np.float32
    f8 = ml_dtypes.float8_e4m3fn
    bf = ml_dtypes.bfloat16
    wq_b = _chunk4(Wq, bf)
    wk_b = _chunk4(Wk, bf)
    wv8 = _chunk4(Wv, f8)
    bqc = np.ascontiguousarray(bq.reshape(DC, 128).T, dtype=f32)
    gammab = np.ascontiguousarray(
        np.broadcast_to(ln_gamma[None, :], (128, D)), dtype=f32)
    betab = np.ascontiguousarray(
        np.broadcast_to(ln_beta[None, :], (128, D)), dtype=f32)
    kvT = [_chunk4(key_value[b].T, bf) for b in range(B)]
    kvt8 = [_chunk4(key_value[b].T, f8) for b in range(B)]
    in_maps = []
    for core in range(N_CORES):
        b, qb = divmod(core, 4)
        blk = query[b, qb * QBLK:(qb + 1) * QBLK, :]
        in_maps.append({
            "xqT": _chunk4(blk.T, bf),
            "xqr": np.ascontiguousarray(blk + bv[None, :], dtype=f32),
            "kvT": kvT[b], "kvt8": kvt8[b],
            "wq": wq_b, "wk": wk_b, "wv8": wv8,
            "bqc": bqc,
            "gammab": gammab, "betab": betab,
        })
    return in_maps


def kernel(query, key_value, Wq, bq, Wk, bk, Wv, bv, ln_gamma, ln_beta,
           _trace=False, _trace_kwargs=None):
    args = [np.asarray(a, dtype=np.float32) for a in
            (query, key_value, Wq, bq, Wk, bk, Wv, bv, ln_gamma, ln_beta)]
    nc = _get_program()
    in_maps = _make_in_maps(*args)
    res = bass_utils.run_bass_kernel_spmd(
        nc, in_maps, core_ids=list(range(N_CORES)), trace=_trace,
        **(_trace_kwargs or {}))
    out = np.empty((B, N_Q, D), np.float32)
    for core in range(N_CORES):
        b, qb = divmod(core, 4)
        out[b, qb * QBLK:(qb + 1) * QBLK, :] = res.results[core]["out"]
    if _trace:
        return out, res
    return out
